# revision 3
# baseline (speedup 1.0000x reference)
"""CondConv (per-sample expert-mixed 3x3 conv) + BatchNorm(batch stats) + ReLU6.

Self-contained Trainium2 Bass kernel, SPMD over 8 NeuronCores.

The axon-tunneled dispatch is transfer-bound (~80 MB/s host<->device), so the
design minimizes bytes on the wire and removes every cross-call stall:
  - x ships as bf16, host-padded to (B, 64, 114, 114)  (6.65 MB/core);
  - expert kernels are combined per sample on host (75 MFLOP sgemm) and ship
    as bf16 in PE slot layout (0.4 MB/core);
  - each core returns its conv output quantized to uint8 with a per
    (sample, channel) scale derived from the on-device abs-max (3.2 MB/core),
    plus a tiny (128, 6) stats tensor: per-(sample, channel) sum, sum of
    squares, and the quant scale;
  - there is NO device-side collective: BatchNorm batch stats are reduced on
    host from the per-core stats (3 KB total) and the BN affine + ReLU6 is
    fused into the host-side uint8 dequantization (one fp32 FMA + clip).
    This keeps each core's NEFF independent, so no core waits on another's
    input transfer.
  - dispatch bypasses run_bass_kernel_spmd's per-call retrace: the shard_map
    jit is built once and cached; the NEFF writes every output element, so the
    "zero output" operands the bass_exec custom call expects are satisfied by
    persistent device-resident buffers (no donation, no per-call upload).

Compute (per core, 4 samples, ~209 us cost model):
  - Each sample's quarter-image lives in a (128, 3420) bf16 tile: partitions
    0-63 hold 30 padded rows, partitions 64-127 the same data shifted one row,
    so the dy=0/dy=1 tap pairs contract as single K=128 matmuls (3 pair slots +
    3 K=64 singles = 6 PE slots per chunk instead of 9).  The two samples of
    a pair run concurrently in PE column groups 0/64 (tile_position).
  - PSUM chunks (4 output rows) accumulate the 6 slots, then ScalarE copies
    them to an SBUF-resident output with a free per-channel accum_out sum;
    VectorE squares the copy for sum(x^2) and a reduce_max of the squares
    feeds the per-sample-channel abs-max for the quant scale.
"""

import numpy as np
import ml_dtypes

import jax
from jax.experimental.shard_map import shard_map
from jax.sharding import Mesh, NamedSharding, PartitionSpec

import concourse.bass as bass
import concourse.bacc as bacc
import concourse.mybir as mybir
import concourse.tile as tile
from concourse.bass2jax import (
    _bass_exec_p,
    install_neuronx_cc_hook,
    partition_id_tensor,
)

F32 = mybir.dt.float32
BF16 = mybir.dt.bfloat16
U8 = mybir.dt.uint8
ALU = mybir.AluOpType
ACTF = mybir.ActivationFunctionType
BF16NP = ml_dtypes.bfloat16

B, E, CIN, COUT, KK, H, W = 32, 8, 64, 64, 3, 112, 112
NCORES = 8
BL = B // NCORES          # 4 samples per core
NPAIR = BL // 2           # 2 sample pairs per core
HP, WP = H + 2, W + 2     # 114, 114 padded image
HWO = H * W               # 12544 output pixels per (sample, channel)
QROWS = 28                # output rows per quarter
NQ = H // QROWS           # 4 quarters
CROWS = 4                 # output rows per PSUM chunk
NJ = QROWS // CROWS       # 7 chunks per quarter
NSLOT = 6                 # 3 K=128 tap-pairs (dy 0&1) + 3 K=64 singles (dy=2)
NCHPP = NQ * NJ           # 28 psum chunks per pair
NCHUNK = NPAIR * NCHPP    # 56 psum chunks
BN_EPS = 1e-5
QMAX = 127.0              # quant: q = out * (QMAX/absmax) + 128.5, truncated


def _build_program():
    nc = bacc.Bacc(
        "TRN2",
        target_bir_lowering=False,
        debug=False,
        num_devices=NCORES,
    )

    xp = nc.dram_tensor("xp", [BL, CIN, HP, WP], BF16, kind="ExternalInput").ap()
    wt = nc.dram_tensor("wt", [128, BL * NSLOT * COUT], BF16, kind="ExternalInput").ap()
    yq = nc.dram_tensor("yq", [BL, COUT, H, W], U8, kind="ExternalOutput").ap()
    st = nc.dram_tensor("st", [128, 6], F32, kind="ExternalOutput").ap()

    # (pair, (h c) = 128, spatial) view of the output
    yq_v = yq.rearrange("(pr h) c r w -> pr (h c) (r w)", h=2)

    with tile.TileContext(nc, num_cores=NCORES) as tc:
        _kernel_body(nc, tc, xp, wt, yq_v, st)

    nc.compile()
    return nc


def _kernel_body(nc, tc, xp_v, wt, yq_v, st):
    with (
        tc.tile_pool(name="const", bufs=1) as cpool,
        tc.tile_pool(name="xin", bufs=2) as xpool,
        tc.tile_pool(name="wtmp", bufs=2) as wpool,
        tc.tile_pool(name="norm", bufs=2) as npool,
        tc.tile_pool(name="psum", bufs=8, space="PSUM") as ppool,
    ):
        # ---- persistent SBUF state ----
        wts_bf = cpool.tile([128, BL * NSLOT * COUT], BF16)  # combined weights
        out_sb = cpool.tile([128, NPAIR * HWO], F32)      # conv output, SBUF resident
        sums = cpool.tile([128, NCHUNK], F32)             # per-chunk sum(x)
        sumsqs = cpool.tile([128, NCHUNK], F32)           # per-chunk sum(x^2)
        mxsqs = cpool.tile([128, NCHUNK], F32)            # per-chunk max(x^2)

        nc.sync.dma_start(wts_bf[:, :], wt)

        # ---- conv: 6 matmul slots per 4-row chunk, 2 PE column groups ----
        FL = 30 * WP  # 3420
        SH = FL - WP  # 3306 valid shifted elements
        ch = 0
        for pr in range(NPAIR):
            for q in range(NQ):
                xts = []
                for h in range(2):
                    xt = xpool.tile([128, FL], BF16, name=f"xt{h}", tag=f"xt{h}")
                    nc.sync.dma_start(
                        xt[0:64, :].rearrange("p (r w) -> p r w", w=WP),
                        xp_v[2 * pr + h, :, q * QROWS:q * QROWS + 30, :],
                    )
                    nc.sync.dma_start(xt[64:128, 0:SH], xt[0:64, WP:FL])
                    xts.append(xt)
                for j in range(NJ):
                    n6 = 456 if j < NJ - 1 else 454
                    ps = ppool.tile([128, 456], F32)
                    for slot in range(NSLOT):
                        pair = slot < 3
                        dx = slot if pair else slot - 3
                        base = (CROWS * j + (0 if pair else 2)) * WP + dx
                        n = 456 if pair else n6
                        kp = 128 if pair else 64
                        for h in range(2):
                            wsl = wts_bf[
                                0:kp,
                                ((2 * pr + h) * NSLOT + slot) * COUT:
                                ((2 * pr + h) * NSLOT + slot + 1) * COUT,
                            ]
                            nc.tensor.matmul(
                                ps[64 * h:64 * h + 64, 0:n],
                                lhsT=wsl,
                                rhs=xts[h][0:kp, base:base + n],
                                start=(slot == 0),
                                stop=(slot == NSLOT - 1),
                                tile_position=(0, 64 * h),
                            )
                    valid = ps[:, 0:456].rearrange("p (r w) -> p r w", w=WP)[:, :, 0:W]
                    ys = (q * QROWS + CROWS * j) * W
                    dest = out_sb[:, pr * HWO + ys:pr * HWO + ys + CROWS * W]
                    nc.scalar.activation(
                        dest.rearrange("p (r w) -> p r w", w=W),
                        valid,
                        ACTF.Copy,
                        accum_out=sums[:, ch:ch + 1],
                    )
                    sqs = wpool.tile([128, CROWS * W], F32)
                    nc.vector.scalar_tensor_tensor(
                        sqs[:, :],
                        dest,
                        0.0,
                        dest,
                        op0=ALU.bypass,
                        op1=ALU.mult,
                        accum_out=sumsqs[:, ch:ch + 1],
                    )
                    nc.vector.reduce_max(
                        mxsqs[:, ch:ch + 1], sqs[:, :], axis=mybir.AxisListType.X
                    )
                    ch += 1

        # ---- per-(partition, pair) stats: sum, sumsq, quant scale ----
        st_t = cpool.tile([128, 6], F32)
        m2 = cpool.tile([128, 2], F32)
        rt = cpool.tile([128, 2], F32)
        sc_t = cpool.tile([128, 2], F32)
        for pr in range(NPAIR):
            cs = slice(pr * NCHPP, (pr + 1) * NCHPP)
            nc.vector.reduce_sum(
                st_t[:, pr:pr + 1], sums[:, cs], axis=mybir.AxisListType.X
            )
            nc.vector.reduce_sum(
                st_t[:, 2 + pr:3 + pr], sumsqs[:, cs], axis=mybir.AxisListType.X
            )
            nc.vector.reduce_max(
                m2[:, pr:pr + 1], mxsqs[:, cs], axis=mybir.AxisListType.X
            )
        nc.vector.tensor_scalar(m2[:, :], m2[:, :], 1e-30, None, op0=ALU.max)
        nc.scalar.activation(rt[:, :], m2[:, :], ACTF.Sqrt)
        nc.vector.reciprocal(sc_t[:, :], rt[:, :])
        nc.vector.tensor_scalar(sc_t[:, :], sc_t[:, :], QMAX, None, op0=ALU.mult)
        nc.vector.tensor_copy(st_t[:, 4:6], sc_t[:, :])
        nc.sync.dma_start(st, st_t[:, :])

        # ---- quantize: q = out * sc + 128.5, clamp, to u8 ----
        bias_t = cpool.tile([128, 1], F32)
        nc.vector.tensor_scalar(
            bias_t[:, :], sc_t[:, 0:1], 0.0, 128.5, op0=ALU.mult, op1=ALU.add
        )
        NS = 1568  # spatial chunk; 8 chunks per (pair half)
        for pr in range(NPAIR):
            for sc in range(HWO // NS):
                src = out_sb[:, pr * HWO + sc * NS:pr * HWO + (sc + 1) * NS]
                t1 = npool.tile([128, NS], F32)
                nc.scalar.activation(
                    t1[:, :], src, ACTF.Identity,
                    bias=bias_t[:, :], scale=sc_t[:, pr:pr + 1],
                )
                tq = npool.tile([128, NS], U8)
                nc.vector.tensor_scalar(
                    tq[:, :], t1[:, :], 0.5, 255.49, op0=ALU.max, op1=ALU.min
                )
                nc.sync.dma_start(yq_v[pr, :, sc * NS:(sc + 1) * NS], tq[:, :])


def _prepare_inputs(x, routing_weight, experts):
    """Host-side layout prep: pad+cast x, sgemm weight combine, slot layout."""
    x = np.ascontiguousarray(x, dtype=np.float32)
    routing_weight = np.ascontiguousarray(routing_weight, dtype=np.float32)
    experts = np.ascontiguousarray(experts, dtype=np.float32)

    xp = np.zeros((B, CIN, HP, WP), dtype=BF16NP)
    xp[:, :, 1:1 + H, 1:1 + W] = x

    # Combine expert kernels per sample: (B, Cout, Cin, K, K), fp32 sgemm.
    kb = (routing_weight @ experts.reshape(E, -1)).reshape(B, COUT, CIN, KK, KK)
    kx = np.transpose(kb, (2, 0, 3, 4, 1))  # (ci, b, dy, dx, co)
    # slot layout: slots 0-2 are K=128 tap pairs (dy = p//64, dx = slot);
    # slots 3-5 are K=64 singles (dy=2, dx = slot-3; upper half zero).
    wt_full = np.zeros((128, B, NSLOT, COUT), dtype=np.float32)
    wt_full[0:64, :, 0:3, :] = kx[:, :, 0]
    wt_full[64:128, :, 0:3, :] = kx[:, :, 1]
    wt_full[0:64, :, 3:6, :] = kx[:, :, 2]
    # global (8*128, BL*NSLOT*COUT): core c's rows are wt_full[:, 4c:4c+4]
    wt_g = np.ascontiguousarray(
        wt_full.reshape(128, NCORES, BL, NSLOT, COUT)
        .transpose(1, 0, 2, 3, 4)
        .reshape(NCORES * 128, BL * NSLOT * COUT)
        .astype(BF16NP)
    )
    return xp, wt_g


_EXEC = None  # (jitted_fn, dummy_out_operands)


def _get_exec():
    global _EXEC
    if _EXEC is None:
        install_neuronx_cc_hook()
        nc = _build_program()

        partition_name = (
            nc.partition_id_tensor.name if nc.partition_id_tensor else None
        )
        in_names = []
        out_names = []
        out_avals = []
        for alloc in nc.m.functions[0].allocations:
            if not isinstance(alloc, mybir.MemoryLocationSet):
                continue
            name = alloc.memorylocations[0].name
            if alloc.kind == "ExternalInput":
                if name != partition_name:
                    in_names.append(name)
            elif alloc.kind == "ExternalOutput":
                out_names.append(name)
                shape = tuple(alloc.tensor_shape)
                dtype = mybir.dt.np(alloc.dtype)
                out_avals.append(jax.core.ShapedArray(shape, dtype))
        n_params = len(in_names)
        n_outs = len(out_avals)
        in_names = in_names + out_names
        if partition_name is not None:
            in_names.append(partition_name)

        def _body(*args):
            operands = list(args)
            if partition_name is not None:
                operands.append(partition_id_tensor())
            outs = _bass_exec_p.bind(
                *operands,
                out_avals=tuple(out_avals),
                in_names=tuple(in_names),
                out_names=tuple(out_names),
                lowering_input_output_aliases=(),
                sim_require_finite=True,
                sim_require_nnan=True,
                nc=nc,
            )
            return tuple(outs)

        devices = jax.devices()[:NCORES]
        mesh = Mesh(np.asarray(devices), ("core",))
        pspec = PartitionSpec("core")
        sharded = jax.jit(
            shard_map(
                _body,
                mesh=mesh,
                in_specs=(pspec,) * (n_params + n_outs),
                out_specs=(pspec,) * n_outs,
                check_rep=False,
            ),
            keep_unused=True,
        )
        # The NEFF writes every element of both outputs, so the "pre-zeroed
        # output" operands never influence the result — persistent
        # device-resident buffers avoid a per-call host->device upload.
        out_sharding = NamedSharding(mesh, pspec)
        dummies = [
            jax.device_put(
                np.zeros((NCORES * a.shape[0], *a.shape[1:]), a.dtype),
                out_sharding,
            )
            for a in out_avals
        ]
        _EXEC = (sharded, dummies, out_avals)
    return _EXEC


def run_on_hw(xp_g, wt_g):
    sharded, dummies, out_avals = _get_exec()
    yq_all, st_all = sharded(xp_g, wt_g, *dummies)
    return np.asarray(yq_all), np.asarray(st_all)


def kernel(x, routing_weight, experts, gamma, beta):
    gamma = np.asarray(gamma, dtype=np.float32)
    beta = np.asarray(beta, dtype=np.float32)
    xp_g, wt_g = _prepare_inputs(x, routing_weight, experts)
    yq, st = run_on_hw(xp_g, wt_g)

    # st[c, p, :] with p = 64*h + channel, cols [sum0, sum1, sq0, sq1, s0, s1]
    stv = st.reshape(NCORES, 2, 64, 6)            # (core, h, channel, col)
    # sample index s = 4*core + 2*pr + h
    sums = stv[:, :, :, 0:2].transpose(0, 3, 1, 2).reshape(B, 64)
    sqs = stv[:, :, :, 2:4].transpose(0, 3, 1, 2).reshape(B, 64)
    qsc = stv[:, :, :, 4:6].transpose(0, 3, 1, 2).reshape(B, 64)

    ntot = float(B * HWO)
    mu = sums.sum(axis=0) / ntot                  # (64,)
    ex2 = sqs.sum(axis=0) / ntot
    var = ex2 - mu * mu
    g = gamma / np.sqrt(var + BN_EPS)             # (64,)

    # y = (deq - mu) * g + beta, deq = (q - 128) / s
    A = (g[None, :] / qsc).astype(np.float32)     # (B, 64)
    Bc = (beta[None, :] - mu[None, :] * g[None, :] - 128.0 * A).astype(np.float32)

    y = yq.astype(np.float32)
    y *= A[:, :, None, None]
    y += Bc[:, :, None, None]
    np.clip(y, 0.0, 6.0, out=y)
    return y
